# revision 52
# baseline (speedup 1.0000x reference)
"""Trainium2 Bass kernel for nn_AttentionLayers (B=64, L=1024, H=512, E=2H=1024).

  context[b] = softmax_l( relu(cat(hidden[b], enc[b,l]) @ W_attn + b_attn) @ W_v ) @ enc[b]

Strategy (data-parallel over batch, 8 batches per core on 8 cores):
  - split W_attn into W1 (hidden part, [512,512]) and W2 (encoder part, [1024,512]).
  - per core, precompute hbT[h, b] = (hidden @ W1 + b_attn).T once on TensorE (tiny),
    then hbw = hbT * |wv| (per-h fold of the attention vector, see below).
  - per batch, the dominant matmul zT[h, l] = sum_k W2[k,h] * encT[k,l] runs in MIXED
    precision along k: the first 4 k-tiles in bf16 (1 matmul each), the last 4 k-tiles
    as fp8e4m3 with MatmulPerfMode.DoubleRow (2 k-tiles per instruction) -> 6 PE
    instructions per 512-wide chunk instead of 8.  All four operands are pre-scaled by
    powers of two (enc*16, W2*128) so both halves accumulate in one PSUM group; the
    2^-11 unscale is folded into the evacuation.  fp8 on half the contraction keeps the
    final rel err ~1.7e-2 (< 2e-2 gate; full fp8 would be 2.39e-2).
  - |wv| is folded into the PSUM evacuation (ScalarE: y' = relu(z*|wv|*2^-11 + hb*|wv|)
    with per-partition scale/bias APs), so att[1, l] is just a +-1-column matmul
    reduction on the PE: att = sum_ht sgn_ht^T @ y'_ht.  This removes the whole
    VectorE partial-product tree of the previous version.
  - softmax: logits are bounded (|att| < ~8) so exp runs WITHOUT the usual
    max-subtraction (ScalarE exp+accum straight from PSUM); w transposed to
    columns via PE transposes.
  - ctx[1, e] = sum_l w[l] * enc_nat[l, e]: VectorE scale+tree over the natural-layout
    bf16 copy of enc, with the 128-partition reduction as ones-matmuls on the PE
    (direct PE matmuls for the last batches to shorten the kernel tail).
  - enc is supplied from the host in BOTH layouts, pre-packed partition-major:
    transposed [e, l] split into a bf16 half (k-tiles 0-3, scaled x16) and an fp8 half
    (k-tiles 4-7, scaled x16), and natural [l, e] in bf16 (unscaled, for ctx).
"""

import sys

for _p in ("/opt/trn_rl_repo",):
    if _p not in sys.path:
        sys.path.insert(0, _p)

import numpy as np
import ml_dtypes

BF16 = ml_dtypes.bfloat16
F8E4 = ml_dtypes.float8_e4m3

N_CORES = 8
B, L, H = 64, 1024, 512
E = 2 * H            # 1024
NB = B // N_CORES    # 8 batches per core
KT = E // 128        # 8 k-tiles over encoder feature dim
KB = 4               # k-tiles 0..KB-1 in bf16
KF = KT - KB         # k-tiles KB..KT-1 in fp8 (DoubleRow pairs)
HT = H // 128        # 4 tiles over hidden dim
LT = L // 128        # 8 l-tiles

# power-of-two scales (exact in bf16/fp8) for the z matmul operands
ENC_SCALE = 16.0     # |enc| <= ~6   -> |enc*16| <= ~96  < 240 (e4m3 max)
W2_SCALE = 128.0     # |W2| <= ~0.15 -> |W2*128| <= ~20  < 240
Z_UNSCALE = 1.0 / (ENC_SCALE * W2_SCALE)  # 2^-11, folded into the evac scale

_CACHE = {}


def _build_program():
    import concourse.tile as tile
    from concourse import bacc, mybir
    from contextlib import ExitStack

    f32 = mybir.dt.float32
    bf = mybir.dt.bfloat16
    f8 = mybir.dt.float8e4
    AF = mybir.ActivationFunctionType
    DR = mybir.MatmulPerfMode.DoubleRow

    nc = bacc.Bacc("TRN2", target_bir_lowering=False, debug=False, enable_asserts=False)

    # all inputs are packed partition-major on the host: row p holds everything
    # partition p needs, contiguously.
    enc_nat = nc.dram_tensor("enc_nat", [NB * 128, LT * E], bf, kind="ExternalInput").ap()
    enc_trb = nc.dram_tensor("enc_trb", [NB * 128, KB * L], bf, kind="ExternalInput").ap()
    enc_tr8 = nc.dram_tensor("enc_tr8", [NB * 128, KF * L], f8, kind="ExternalInput").ap()
    w2b_d = nc.dram_tensor("w2b", [128, KB * H], bf, kind="ExternalInput").ap()
    w28_d = nc.dram_tensor("w28", [128, KF * H], f8, kind="ExternalInput").ap()
    # merged small consts: [w1 | hidT | sgn] bf16 and [bvec | wvs | wva] f32
    cb_d = nc.dram_tensor(
        "cbf", [128, HT * H + HT * NB + HT], bf, kind="ExternalInput").ap()
    cf_d = nc.dram_tensor("cf32", [128, 3 * HT], f32, kind="ExternalInput").ap()
    ctx_d = nc.dram_tensor("ctx", [NB, E], f32, kind="ExternalOutput").ap()

    with tile.TileContext(nc) as tc, ExitStack() as ctx:
        consts = ctx.enter_context(tc.tile_pool(name="consts", bufs=1))
        nat_pool = ctx.enter_context(tc.tile_pool(name="nat", bufs=2))
        trb_pool = ctx.enter_context(tc.tile_pool(name="trb", bufs=3))
        tr8_pool = ctx.enter_context(tc.tile_pool(name="tr8", bufs=3))
        en_pool = ctx.enter_context(tc.tile_pool(name="en", bufs=2))
        sm_pool = ctx.enter_context(tc.tile_pool(name="sm", bufs=2))
        out_pool = ctx.enter_context(tc.tile_pool(name="outp", bufs=2))
        ypool = ctx.enter_context(tc.tile_pool(name="ypool", bufs=2))
        zps = ctx.enter_context(tc.tile_pool(name="zps", bufs=2, space="PSUM"))
        rowps = ctx.enter_context(tc.tile_pool(name="rowps", bufs=2, space="PSUM"))
        smallps = ctx.enter_context(tc.tile_pool(name="smallps", bufs=1, space="PSUM"))

        # ---- PE warm-up: dummy matmuls with no DMA deps keep the PE busy while
        # the first loads land, so HAM un-throttles before the real work ----
        N_WARMUP = 14
        wup = consts.tile([128, 128], bf)
        nc.vector.memset(wup[:, :], 0.0)
        wup_ps = smallps.tile([128, 128], f32, tag="wup")
        for _ in range(N_WARMUP):
            nc.tensor.matmul(wup_ps, wup[:, :], wup[:, :], start=True, stop=True)

        # ---- startup loads: interleave W2 blocks with batch-0 enc k-tiles in
        # consumption order so the z matmuls can start as soon as possible ----
        w2b_sb = consts.tile([128, KB, H], bf)
        w28_sb = consts.tile([128, KF, H], f8)
        enc_tb0 = trb_pool.tile([128, KB, L], bf, tag="enc_tb")
        enc_t80 = tr8_pool.tile([128, KF, L], f8, tag="enc_t8")

        # sync ring: batch-0 tiles in k-outer consumption order, few large
        # issues (each dma_start costs ~0.7us of issue time on its ring)
        nc.sync.dma_start(w2b_sb[:, 0, :], w2b_d[:, 0:H])
        nc.sync.dma_start(enc_tb0[:, 0, :], enc_trb[0:128, 0:L])
        nc.sync.dma_start(w2b_sb[:, 1:KB, :], w2b_d[:, H:KB * H])
        nc.sync.dma_start(enc_tb0[:, 1, :], enc_trb[0:128, L:2 * L])
        nc.sync.dma_start(enc_tb0[:, 2, :], enc_trb[0:128, 2 * L:3 * L])
        nc.sync.dma_start(enc_tb0[:, 3, :], enc_trb[0:128, 3 * L:4 * L])
        nc.sync.dma_start(w28_sb, w28_d[:, :])
        nc.sync.dma_start(enc_t80[:, 0:2, :], enc_tr8[0:128, 0:2 * L])
        nc.sync.dma_start(enc_t80[:, 2:4, :], enc_tr8[0:128, 2 * L:4 * L])
        # scalar ring (idle this early): merged small consts, then batch-1
        cf_sb = consts.tile([128, 3 * HT], f32)
        nc.scalar.dma_start(cf_sb, cf_d[:, :])
        cb_sb = consts.tile([128, HT * H + HT * NB + HT], bf)
        nc.scalar.dma_start(cb_sb, cb_d[:, :])
        w1_sb = cb_sb[:, 0:HT * H].rearrange("p (t h) -> p t h", t=HT)
        hidT_sb = cb_sb[:, HT * H:HT * H + HT * NB].rearrange(
            "p (t n) -> p t n", t=HT)
        sgn_sb = cb_sb[:, HT * H + HT * NB:]
        b_sb = cf_sb[:, 0:HT]
        wvs_sb = cf_sb[:, HT:2 * HT]
        wva_sb = cf_sb[:, 2 * HT:3 * HT]
        enc_tb1 = trb_pool.tile([128, KB, L], bf, tag="enc_tb", name="enc_tb1")
        enc_t81 = tr8_pool.tile([128, KF, L], f8, tag="enc_t8", name="enc_t81")
        ident = consts.tile([1, 1], f32)
        nc.vector.memset(ident[:, :], 1.0)
        ones_col = consts.tile([128, 1], bf)
        nc.vector.memset(ones_col[:, :], 1.0)

        # hbw[h, b] = (hidden @ W1 + b_attn).T * |wv|  — emitted lazily (after a
        # couple of batch-0 z groups) so its weight loads don't stall the PE stream.
        hbw_sb = consts.tile([128, HT, NB], f32)

        def emit_hb():
            for ht in range(HT):
                hb_ps = smallps.tile([128, NB], f32, tag="hb")
                for k in range(HT):
                    nc.tensor.matmul(
                        hb_ps,
                        w1_sb[:, k, ht * 128:(ht + 1) * 128],
                        hidT_sb[:, k, :],
                        start=(k == 0),
                        stop=(k == HT - 1),
                    )
                nc.scalar.activation(
                    hbw_sb[:, ht, :], hb_ps, AF.Identity,
                    bias=b_sb[:, ht:ht + 1], scale=1.0,
                )
                nc.vector.tensor_scalar_mul(
                    hbw_sb[:, ht, :], hbw_sb[:, ht, :], wva_sb[:, ht:ht + 1]
                )

        # ---- per-batch pipeline, software-pipelined across batches:
        # batch b's att(lc1) / softmax / w-transposes / ctx work is deferred and
        # spread over several slots between batch b+1's z groups so no PE
        # instruction ever sits in the queue behind unfinished Scalar/DVE work ----
        def make_deferred(b, att_ps, enc_n, yt, last):
            final = b == NB - 1

            def fillers(n):
                for _ in range(n):
                    nc.tensor.matmul(
                        wup_ps, wup[:, :], wup[:, :], start=True, stop=True
                    )

            st = sm_pool.tile([1, 5], f32, tag="softstate", name=f"st_{b}")
            # [1:3]=exp partial sums, [3]=sumexp, [4]=1/sumexp
            w_row = sm_pool.tile([1, L], f32, name=f"w_row_{b}")
            state = {}

            def d0_att_lc1():
                if final:
                    fillers(10)
                for ht in range(HT):
                    nc.tensor.matmul(
                        att_ps[:, 512:1024],
                        sgn_sb[:, ht:ht + 1],
                        yt[:, ht, 512:1024],
                        start=(ht == 0),
                        stop=(ht == HT - 1),
                    )

            def d0b_exp():
                # logits are bounded (|att| < ~8): f32 exp is safe without the
                # usual max-subtraction, which keeps VectorE off the chain
                for lc2 in range(2):
                    ls2 = lc2 * 512
                    nc.scalar.activation(
                        w_row[:, ls2:ls2 + 512], att_ps[:, ls2:ls2 + 512], AF.Exp,
                        bias=0.0, scale=1.0,
                        accum_out=st[:, 1 + lc2:2 + lc2],
                    )
                nc.vector.tensor_reduce(
                    st[:, 3:4], st[:, 1:3], axis=mybir.AxisListType.X,
                    op=mybir.AluOpType.add,
                )
                nc.vector.reciprocal(st[:, 4:5], st[:, 3:4])

            def d0c_transpose():
                if final:
                    fillers(16)
                wT_ps = smallps.tile([128, LT], f32, tag="wup")
                for j in range(8):
                    nc.tensor.transpose(
                        wT_ps[:, j:j + 1], w_row[:, j * 128:(j + 1) * 128],
                        ident[:, :],
                    )
                if last:
                    wcb = sm_pool.tile([128, LT], bf, name=f"wcb_{b}", tag="wcb")
                    nc.vector.tensor_copy(wcb[:, 0:4], wT_ps[:, 0:4])
                    nc.vector.tensor_copy(wcb[:, 4:8], wT_ps[:, 4:8])
                    state["wcb"] = wcb
                else:
                    wc = sm_pool.tile([128, LT], f32, name=f"wcf_{b}", tag="wcf")
                    nc.vector.tensor_copy(wc, wT_ps)
                    state["wc"] = wc

            def d2_ctx(half):
                if last:
                    # kernel tail: PE is idle here, and the VectorE tree would
                    # serialize — direct PE matmuls, pipelined by lt-halves so
                    # the first four weight columns start the reduction early
                    ctx_ps = state.get("ctx_ps")
                    if ctx_ps is None:
                        ctx_ps = rowps.tile(
                            [1, E], f32, tag="rowps", name=f"ctx_ps_{b}"
                        )
                        state["ctx_ps"] = ctx_ps
                    for lt in range(4 * half, 4 * half + 4):
                        for es in (0, 512):
                            nc.tensor.matmul(
                                ctx_ps[:, es:es + 512],
                                state["wcb"][:, lt:lt + 1],
                                enc_n[:, lt, es:es + 512],
                                start=(lt == 0),
                                stop=(lt == LT - 1),
                            )
                    if half == 1:
                        ctx_sb = out_pool.tile([1, E], f32, name=f"ctx_sb_{b}")
                        nc.vector.tensor_scalar_mul(ctx_sb, ctx_ps, st[:, 4:5])
                        nc.sync.dma_start(ctx_d[b:b + 1, :], ctx_sb)
                    return
                # ctx partial products on VectorE: ct_lt = w[lt-chunk] * enc_nat
                # (per-partition scalar), pairwise-summed down to one [128, E]
                # tile; the 128-partition reduction is two ones-matmuls on PE
                wc = state["wc"]
                s_tiles = []
                for i in range(4):
                    lt = half * 4 + i
                    ct = ypool.tile(
                        [128, E], bf, name=f"ct{i}", tag=f"ct{i}", bufs=1
                    )
                    nc.vector.tensor_scalar_mul(
                        ct, enc_n[:, lt, :], wc[:, lt:lt + 1]
                    )
                    s_tiles.append(ct)
                s0 = ypool.tile(
                    [128, E], bf, name=f"cs{half}", tag=f"cs{half}", bufs=1
                )
                nc.vector.tensor_add(s0, s_tiles[0], s_tiles[1])
                s1 = ypool.tile(
                    [128, E], bf, name=f"cs{half}b", tag=f"cs{half}b", bufs=1
                )
                nc.vector.tensor_add(s1, s_tiles[2], s_tiles[3])
                state[f"s{half}"] = (s0, s1)

            def d3_ctx_adds():
                if last:
                    return
                a0, a1 = state["s0"]
                b0, b1 = state["s1"]
                t0 = ypool.tile([128, E], bf, name="cty0", tag="ct0", bufs=1)
                nc.vector.tensor_add(t0, a0, a1)
                t1 = ypool.tile([128, E], bf, name="cty1", tag="ct1", bufs=1)
                nc.vector.tensor_add(t1, b0, b1)
                cty = ypool.tile([128, E], bf, name="cty", tag="ct2", bufs=1)
                nc.vector.tensor_add(cty, t0, t1)
                state["cty"] = cty

            def d4_ctx_out():
                if last:
                    return
                cty = state["cty"]
                ctx_ps = rowps.tile(
                    [1, E], f32, tag="rowps", name=f"ctx_ps_{b}"
                )
                for ec in range(2):
                    es = ec * 512
                    nc.tensor.matmul(
                        ctx_ps[:, es:es + 512],
                        ones_col[:, :],
                        cty[:, es:es + 512],
                        start=True,
                        stop=True,
                    )
                ctx_sb = out_pool.tile([1, E], f32)
                nc.vector.tensor_scalar_mul(ctx_sb, ctx_ps, st[:, 4:5])
                nc.sync.dma_start(ctx_d[b:b + 1, :], ctx_sb)

            return [d0_att_lc1, d0b_exp, lambda: None, d0c_transpose,
                    lambda: d2_ctx(0), lambda: d2_ctx(1), d3_ctx_adds, d4_ctx_out]

        deferred = []
        enc_tb_tiles = {0: enc_tb0, 1: enc_tb1}
        enc_t8_tiles = {0: enc_t80, 1: enc_t81}
        for b in range(NB):
            nb2 = b + 2
            if nb2 < NB:
                tb = trb_pool.tile(
                    [128, KB, L], bf, tag="enc_tb", name=f"enc_tb{nb2}"
                )
                t8 = tr8_pool.tile(
                    [128, KF, L], f8, tag="enc_t8", name=f"enc_t8{nb2}"
                )
                rb = nb2 * 128
                nc.sync.dma_start(tb, enc_trb[rb:rb + 128, :])
                nc.sync.dma_start(t8, enc_tr8[rb:rb + 128, :])
                enc_tb_tiles[nb2] = tb
                enc_t8_tiles[nb2] = t8
            enc_tb = enc_tb_tiles.pop(b)
            enc_t8 = enc_t8_tiles.pop(b)

            def tb_ap(k, ls, enc_tb=enc_tb):
                return enc_tb[:, k, ls:ls + 512]

            def t8_ap(kp, ls, enc_t8=enc_t8):
                return enc_t8[:, 2 * kp:2 * kp + 2, ls:ls + 512]
            enc_n = nat_pool.tile([128, LT, E], bf)

            # y'[h, l] = |wv_h| * relu(energy) — evacuated straight from PSUM
            yt = en_pool.tile([128, HT, L], bf)
            att_ps = None
            pending = []
            gidx = 0

            def emit_evac(zp, lc, ht):
                ls = lc * 512
                nc.scalar.activation(
                    yt[:, ht, ls:ls + 512], zp, AF.Relu,
                    bias=hbw_sb[:, ht, b:b + 1], scale=wvs_sb[:, ht:ht + 1],
                )

            if b == 0:
                # ---- warm start: lc0 in k-outer order over chunk pairs so the
                # PE consumes each k-tile as its DMA lands instead of waiting
                # for the whole contraction ----
                def fill(n):
                    for _ in range(n):
                        nc.tensor.matmul(
                            wup_ps, wup[:, :], wup[:, :], start=True, stop=True
                        )

                for pair in range(2):
                    hts = (2 * pair, 2 * pair + 1)
                    zp_pair = {
                        ht: zps.tile([128, 512], f32, name=f"zp0{ht}", tag="zp")
                        for ht in hts
                    }
                    for k in range(KB):
                        for ht in hts:
                            nc.tensor.matmul(
                                zp_pair[ht],
                                w2b_sb[:, k, ht * 128:(ht + 1) * 128],
                                enc_tb[:, k, 0:512],
                                start=(k == 0),
                                stop=False,
                            )
                        if pair == 0:
                            fill(8)
                    for kp in range(KF // 2):
                        for ht in hts:
                            nc.tensor.matmul(
                                zp_pair[ht],
                                w28_sb[:, 2 * kp:2 * kp + 2,
                                       ht * 128:(ht + 1) * 128],
                                enc_t8[:, 2 * kp:2 * kp + 2, 0:512],
                                start=False,
                                stop=(kp == KF // 2 - 1),
                                perf_mode=DR,
                            )
                    if pair == 0:
                        # hb weights (w1/hidT) have landed by now; trace it
                        # before the first evacuation reads hbw
                        emit_hb()
                        # batch-1 loads issue only now (scalar ring) so they
                        # don't steal HBM bandwidth from batch-0's startup
                        nc.scalar.dma_start(enc_tb1, enc_trb[128:256, :])
                        nc.scalar.dma_start(enc_t81, enc_tr8[128:256, :])
                    for ht in hts:
                        emit_evac(zp_pair[ht], 0, ht)
                nc.scalar.dma_start(enc_n, enc_nat[0:128, :])

            for lc in range(2):
                if b == 0 and lc == 0:
                    continue
                ls = lc * 512
                for ht in range(HT):
                    zp = zps.tile([128, 512], f32, tag="zp")
                    for k in range(KB):
                        nc.tensor.matmul(
                            zp,
                            w2b_sb[:, k, ht * 128:(ht + 1) * 128],
                            tb_ap(k, ls),
                            start=(k == 0),
                            stop=False,
                        )
                    for kp in range(KF // 2):
                        nc.tensor.matmul(
                            zp,
                            w28_sb[:, 2 * kp:2 * kp + 2,
                                   ht * 128:(ht + 1) * 128],
                            t8_ap(kp, ls),
                            start=False,
                            stop=(kp == KF // 2 - 1),
                            perf_mode=DR,
                        )
                    emit_evac(zp, lc, ht)
                    if lc == 0 and ht == 3:
                        # natural-layout load issued mid-batch on the second
                        # HWDGE ring: needed only by ctx during the next batch,
                        # and issuing it late keeps the z path fed first
                        nc.scalar.dma_start(
                            enc_n, enc_nat[b * 128:(b + 1) * 128, :]
                        )
                    # previous batch's deferred att/softmax/ctx work slots in
                    # between this batch's z groups
                    if gidx < len(deferred):
                        deferred[gidx]()
                    gidx += 1
                    if lc == 1 and ht == 0:
                        # this batch's att(lc0): slack after the lc0 evacuations
                        att_ps = rowps.tile([1, L], f32, tag="rowps")
                        for ht2 in range(HT):
                            nc.tensor.matmul(
                                att_ps[:, 0:512],
                                sgn_sb[:, ht2:ht2 + 1],
                                yt[:, ht2, 0:512],
                                start=(ht2 == 0),
                                stop=(ht2 == HT - 1),
                            )
            deferred = make_deferred(b, att_ps, enc_n, yt,
                                     last=(b == NB - 1))

        # drain the last batch's deferred work
        for fn in deferred:
            fn()

    nc.compile()
    return nc


def _get_program():
    if "nc" not in _CACHE:
        _CACHE["nc"] = _build_program()
    return _CACHE["nc"]


def _pmajor(a, tiles, p=128):
    """[tiles*p, F] -> [p, tiles*F] partition-major packing."""
    t, rem = divmod(a.shape[0], p)
    assert rem == 0 and t == tiles
    f = a.shape[1]
    return np.ascontiguousarray(
        a.reshape(tiles, p, f).transpose(1, 0, 2).reshape(p, tiles * f)
    )


def _prep_in_maps(hidden, encoder_outputs, W_attn, b_attn, W_v):
    hidden = np.asarray(hidden, dtype=np.float32)
    encoder_outputs = np.asarray(encoder_outputs, dtype=np.float32)
    W_attn = np.asarray(W_attn, dtype=np.float32)
    b_attn = np.asarray(b_attn, dtype=np.float32)
    W_v = np.asarray(W_v, dtype=np.float32)

    enc_bf = encoder_outputs.astype(BF16)
    enc_s = encoder_outputs * ENC_SCALE           # scaled copy for the z matmul
    W2s = W_attn[H:] * W2_SCALE
    w2b = _pmajor(np.ascontiguousarray(W2s[:KB * 128]).astype(BF16), KB)
    w28 = _pmajor(np.ascontiguousarray(W2s[KB * 128:]).astype(F8E4), KF)
    w1 = _pmajor(np.ascontiguousarray(W_attn[:H]).astype(BF16), HT)
    bvec = np.ascontiguousarray(b_attn.reshape(HT, 128).T)
    wv = W_v[:, 0]
    wva = np.abs(wv).astype(np.float32)
    wvs = (wva * Z_UNSCALE).astype(np.float32)
    sgn = np.where(wv >= 0, 1.0, -1.0)
    wva = np.ascontiguousarray(wva.reshape(HT, 128).T)
    wvs = np.ascontiguousarray(wvs.reshape(HT, 128).T)
    sgn = np.ascontiguousarray(sgn.reshape(HT, 128).T.astype(BF16))
    cf32 = np.ascontiguousarray(
        np.concatenate([bvec, wvs, wva], axis=1).astype(np.float32))

    in_maps = []
    for c in range(N_CORES):
        sl = slice(c * NB, (c + 1) * NB)
        eb = enc_bf[sl]
        # natural [l, e] rows, partition-major per batch: [NB*128, LT*E]
        nat = np.ascontiguousarray(
            eb.reshape(NB, LT, 128, E).transpose(0, 2, 1, 3)
        ).reshape(NB * 128, LT * E)
        # transposed [e, l] rows, partition-major per batch, split by k-tile
        # precision: bf16 tiles 0..KB-1 and fp8 tiles KB..KT-1 (both scaled)
        et = enc_s[sl].transpose(0, 2, 1)         # [NB, E, L] scaled
        trb = np.ascontiguousarray(
            et[:, :KB * 128].astype(BF16)
            .reshape(NB, KB, 128, L).transpose(0, 2, 1, 3)
        ).reshape(NB * 128, KB * L)
        tr8 = np.ascontiguousarray(
            et[:, KB * 128:].astype(F8E4)
            .reshape(NB, KF, 128, L).transpose(0, 2, 1, 3)
        ).reshape(NB * 128, KF * L)
        hidT = _pmajor(np.ascontiguousarray(hidden[sl].T).astype(BF16), HT)
        cbf = np.ascontiguousarray(
            np.concatenate([w1, hidT, sgn], axis=1).astype(BF16))
        in_maps.append({
            "enc_nat": nat,
            "enc_trb": trb,
            "enc_tr8": tr8,
            "w2b": w2b,
            "w28": w28,
            "cbf": cbf,
            "cf32": cf32,
        })
    return in_maps


def _run(inputs, trace=False, tmpdir=None):
    from concourse.bass_utils import run_bass_kernel_spmd

    nc = _get_program()
    in_maps = _prep_in_maps(**inputs)
    res = run_bass_kernel_spmd(
        nc, in_maps, core_ids=list(range(N_CORES)), trace=trace, tmpdir=tmpdir
    )
    out = np.concatenate(
        [np.asarray(res.results[c]["ctx"]) for c in range(N_CORES)], axis=0
    ).astype(np.float32)
    return out.reshape(B, 1, E), res


def kernel(hidden, encoder_outputs, W_attn, b_attn, W_v):
    out, _ = _run(dict(
        hidden=hidden, encoder_outputs=encoder_outputs,
        W_attn=W_attn, b_attn=b_attn, W_v=W_v,
    ))
    return out


# revision 53
# speedup vs baseline: 1.0358x; 1.0358x over previous
"""Trainium2 Bass kernel for nn_AttentionLayers (B=64, L=1024, H=512, E=2H=1024).

  context[b] = softmax_l( relu(cat(hidden[b], enc[b,l]) @ W_attn + b_attn) @ W_v ) @ enc[b]

Strategy (data-parallel over batch, 8 batches per core on 8 cores):
  - split W_attn into W1 (hidden part, [512,512]) and W2 (encoder part, [1024,512]).
  - per core, precompute hbT[h, b] = (hidden @ W1 + b_attn).T once on TensorE (tiny),
    then hbw = hbT * |wv| (per-h fold of the attention vector, see below).
  - per batch, the dominant matmul zT[h, l] = sum_k W2[k,h] * encT[k,l] runs in MIXED
    precision along k: the first 4 k-tiles in bf16 (1 matmul each), the last 4 k-tiles
    as fp8e4m3 with MatmulPerfMode.DoubleRow (2 k-tiles per instruction) -> 6 PE
    instructions per 512-wide chunk instead of 8.  All four operands are pre-scaled by
    powers of two (enc*16, W2*128) so both halves accumulate in one PSUM group; the
    2^-11 unscale is folded into the evacuation.  fp8 on half the contraction keeps the
    final rel err ~1.7e-2 (< 2e-2 gate; full fp8 would be 2.39e-2).
  - |wv| is folded into the PSUM evacuation (ScalarE: y' = relu(z*|wv|*2^-11 + hb*|wv|)
    with per-partition scale/bias APs), so att[1, l] is just a +-1-column matmul
    reduction on the PE: att = sum_ht sgn_ht^T @ y'_ht.  This removes the whole
    VectorE partial-product tree of the previous version.
  - softmax: logits are bounded (|att| < ~8) so exp runs WITHOUT the usual
    max-subtraction (ScalarE exp+accum straight from PSUM); w transposed to
    columns via PE transposes.
  - ctx[1, e] = sum_l w[l] * enc_nat[l, e]: VectorE scale+tree over the natural-layout
    bf16 copy of enc, with the 128-partition reduction as ones-matmuls on the PE
    (direct PE matmuls for the last batches to shorten the kernel tail).
  - enc is supplied from the host in BOTH layouts, pre-packed partition-major:
    transposed [e, l] split into a bf16 half (k-tiles 0-3, scaled x16) and an fp8 half
    (k-tiles 4-7, scaled x16), and natural [l, e] in bf16 (unscaled, for ctx).
"""

import sys

for _p in ("/opt/trn_rl_repo",):
    if _p not in sys.path:
        sys.path.insert(0, _p)

import numpy as np
import ml_dtypes

BF16 = ml_dtypes.bfloat16
F8E4 = ml_dtypes.float8_e4m3

N_CORES = 8
B, L, H = 64, 1024, 512
E = 2 * H            # 1024
NB = B // N_CORES    # 8 batches per core
KT = E // 128        # 8 k-tiles over encoder feature dim
KB = 4               # k-tiles 0..KB-1 in bf16
KF = KT - KB         # k-tiles KB..KT-1 in fp8 (DoubleRow pairs)
HT = H // 128        # 4 tiles over hidden dim
LT = L // 128        # 8 l-tiles

# power-of-two scales (exact in bf16/fp8) for the z matmul operands
ENC_SCALE = 16.0     # |enc| <= ~6   -> |enc*16| <= ~96  < 240 (e4m3 max)
W2_SCALE = 128.0     # |W2| <= ~0.15 -> |W2*128| <= ~20  < 240
Z_UNSCALE = 1.0 / (ENC_SCALE * W2_SCALE)  # 2^-11, folded into the evac scale

_CACHE = {}


def _build_program():
    import concourse.tile as tile
    from concourse import bacc, mybir
    from contextlib import ExitStack

    f32 = mybir.dt.float32
    bf = mybir.dt.bfloat16
    f8 = mybir.dt.float8e4
    AF = mybir.ActivationFunctionType
    DR = mybir.MatmulPerfMode.DoubleRow

    nc = bacc.Bacc("TRN2", target_bir_lowering=False, debug=False, enable_asserts=False)

    # all inputs are packed partition-major on the host: row p holds everything
    # partition p needs, contiguously.
    enc_nat = nc.dram_tensor("enc_nat", [NB * 128, LT * E], bf, kind="ExternalInput").ap()
    enc_trb = nc.dram_tensor("enc_trb", [NB * 128, KB * L], bf, kind="ExternalInput").ap()
    enc_tr8 = nc.dram_tensor("enc_tr8", [NB * 128, KF * L], f8, kind="ExternalInput").ap()
    w2b_d = nc.dram_tensor("w2b", [128, KB * H], bf, kind="ExternalInput").ap()
    w28_d = nc.dram_tensor("w28", [128, KF * H], f8, kind="ExternalInput").ap()
    # merged small consts: [w1 | hidT | sgn] bf16 and [bvec | wvs | wva] f32
    cb_d = nc.dram_tensor(
        "cbf", [128, HT * H + HT * NB + HT], bf, kind="ExternalInput").ap()
    cf_d = nc.dram_tensor("cf32", [128, 3 * HT], f32, kind="ExternalInput").ap()
    ctx_d = nc.dram_tensor("ctx", [NB, E], f32, kind="ExternalOutput").ap()

    with tile.TileContext(nc) as tc, ExitStack() as ctx:
        consts = ctx.enter_context(tc.tile_pool(name="consts", bufs=1))
        nat_pool = ctx.enter_context(tc.tile_pool(name="nat", bufs=2))
        trb_pool = ctx.enter_context(tc.tile_pool(name="trb", bufs=3))
        tr8_pool = ctx.enter_context(tc.tile_pool(name="tr8", bufs=3))
        en_pool = ctx.enter_context(tc.tile_pool(name="en", bufs=2))
        sm_pool = ctx.enter_context(tc.tile_pool(name="sm", bufs=2))
        out_pool = ctx.enter_context(tc.tile_pool(name="outp", bufs=2))
        ypool = ctx.enter_context(tc.tile_pool(name="ypool", bufs=2))
        zps = ctx.enter_context(tc.tile_pool(name="zps", bufs=2, space="PSUM"))
        rowps = ctx.enter_context(tc.tile_pool(name="rowps", bufs=2, space="PSUM"))
        smallps = ctx.enter_context(tc.tile_pool(name="smallps", bufs=1, space="PSUM"))

        # ---- PE warm-up: dummy matmuls with no DMA deps keep the PE busy while
        # the first loads land, so HAM un-throttles before the real work ----
        N_WARMUP = 19
        wup = consts.tile([128, 128], bf)
        nc.vector.memset(wup[:, :], 0.0)
        wup_ps = smallps.tile([128, 128], f32, tag="wup")
        for _ in range(N_WARMUP):
            nc.tensor.matmul(wup_ps, wup[:, :], wup[:, :], start=True, stop=True)

        # ---- startup loads: interleave W2 blocks with batch-0 enc k-tiles in
        # consumption order so the z matmuls can start as soon as possible ----
        w2b_sb = consts.tile([128, KB, H], bf)
        w28_sb = consts.tile([128, KF, H], f8)
        enc_tb0 = trb_pool.tile([128, KB, L], bf, tag="enc_tb")
        enc_t80 = tr8_pool.tile([128, KF, L], f8, tag="enc_t8")

        # sync ring: batch-0 tiles in k-outer consumption order, few large
        # issues (each dma_start costs ~0.7us of issue time on its ring)
        nc.sync.dma_start(w2b_sb[:, 0, :], w2b_d[:, 0:H])
        nc.sync.dma_start(enc_tb0[:, 0, :], enc_trb[0:128, 0:L])
        nc.sync.dma_start(w2b_sb[:, 1:KB, :], w2b_d[:, H:KB * H])
        nc.sync.dma_start(enc_tb0[:, 1, :], enc_trb[0:128, L:2 * L])
        nc.sync.dma_start(enc_tb0[:, 2, :], enc_trb[0:128, 2 * L:3 * L])
        nc.sync.dma_start(enc_tb0[:, 3, :], enc_trb[0:128, 3 * L:4 * L])
        nc.sync.dma_start(w28_sb, w28_d[:, :])
        nc.sync.dma_start(enc_t80[:, 0:2, :], enc_tr8[0:128, 0:2 * L])
        nc.sync.dma_start(enc_t80[:, 2:4, :], enc_tr8[0:128, 2 * L:4 * L])
        # scalar ring (idle this early): merged small consts, then batch-1
        cf_sb = consts.tile([128, 3 * HT], f32)
        nc.scalar.dma_start(cf_sb, cf_d[:, :])
        cb_sb = consts.tile([128, HT * H + HT * NB + HT], bf)
        nc.scalar.dma_start(cb_sb, cb_d[:, :])
        w1_sb = cb_sb[:, 0:HT * H].rearrange("p (t h) -> p t h", t=HT)
        hidT_sb = cb_sb[:, HT * H:HT * H + HT * NB].rearrange(
            "p (t n) -> p t n", t=HT)
        sgn_sb = cb_sb[:, HT * H + HT * NB:]
        b_sb = cf_sb[:, 0:HT]
        wvs_sb = cf_sb[:, HT:2 * HT]
        wva_sb = cf_sb[:, 2 * HT:3 * HT]
        enc_tb1 = trb_pool.tile([128, KB, L], bf, tag="enc_tb", name="enc_tb1")
        nc.scalar.dma_start(enc_tb1, enc_trb[128:256, :])
        enc_t81 = tr8_pool.tile([128, KF, L], f8, tag="enc_t8", name="enc_t81")
        nc.scalar.dma_start(enc_t81, enc_tr8[128:256, :])
        ident = consts.tile([1, 1], f32)
        nc.vector.memset(ident[:, :], 1.0)
        ones_col = consts.tile([128, 1], bf)
        nc.vector.memset(ones_col[:, :], 1.0)

        # hbw[h, b] = (hidden @ W1 + b_attn).T * |wv|  — emitted lazily (after a
        # couple of batch-0 z groups) so its weight loads don't stall the PE stream.
        hbw_sb = consts.tile([128, HT, NB], f32)

        def emit_hb():
            for ht in range(HT):
                hb_ps = smallps.tile([128, NB], f32, tag="hb")
                for k in range(HT):
                    nc.tensor.matmul(
                        hb_ps,
                        w1_sb[:, k, ht * 128:(ht + 1) * 128],
                        hidT_sb[:, k, :],
                        start=(k == 0),
                        stop=(k == HT - 1),
                    )
                nc.scalar.activation(
                    hbw_sb[:, ht, :], hb_ps, AF.Identity,
                    bias=b_sb[:, ht:ht + 1], scale=1.0,
                )
                nc.vector.tensor_scalar_mul(
                    hbw_sb[:, ht, :], hbw_sb[:, ht, :], wva_sb[:, ht:ht + 1]
                )

        # ---- per-batch pipeline, software-pipelined across batches:
        # batch b's att(lc1) / softmax / w-transposes / ctx work is deferred and
        # spread over several slots between batch b+1's z groups so no PE
        # instruction ever sits in the queue behind unfinished Scalar/DVE work ----
        def make_deferred(b, att_ps, enc_n, yt, last):
            final = b == NB - 1

            def fillers(n):
                for _ in range(n):
                    nc.tensor.matmul(
                        wup_ps, wup[:, :], wup[:, :], start=True, stop=True
                    )

            st = sm_pool.tile([1, 5], f32, tag="softstate", name=f"st_{b}")
            # [1:3]=exp partial sums, [3]=sumexp, [4]=1/sumexp
            w_row = sm_pool.tile([1, L], f32, name=f"w_row_{b}")
            state = {}

            def d0_att_lc1():
                if final:
                    fillers(10)
                for ht in range(HT):
                    nc.tensor.matmul(
                        att_ps[:, 512:1024],
                        sgn_sb[:, ht:ht + 1],
                        yt[:, ht, 512:1024],
                        start=(ht == 0),
                        stop=(ht == HT - 1),
                    )

            def d0b_exp():
                # logits are bounded (|att| < ~8): f32 exp is safe without the
                # usual max-subtraction, which keeps VectorE off the chain
                for lc2 in range(2):
                    ls2 = lc2 * 512
                    nc.scalar.activation(
                        w_row[:, ls2:ls2 + 512], att_ps[:, ls2:ls2 + 512], AF.Exp,
                        bias=0.0, scale=1.0,
                        accum_out=st[:, 1 + lc2:2 + lc2],
                    )
                nc.vector.tensor_reduce(
                    st[:, 3:4], st[:, 1:3], axis=mybir.AxisListType.X,
                    op=mybir.AluOpType.add,
                )
                nc.vector.reciprocal(st[:, 4:5], st[:, 3:4])

            def d0c_transpose():
                if final:
                    fillers(16)
                wT_ps = smallps.tile([128, LT], f32, tag="wup")
                for j in range(8):
                    nc.tensor.transpose(
                        wT_ps[:, j:j + 1], w_row[:, j * 128:(j + 1) * 128],
                        ident[:, :],
                    )
                if last:
                    wcb = sm_pool.tile([128, LT], bf, name=f"wcb_{b}", tag="wcb")
                    nc.vector.tensor_copy(wcb[:, 0:4], wT_ps[:, 0:4])
                    nc.vector.tensor_copy(wcb[:, 4:8], wT_ps[:, 4:8])
                    state["wcb"] = wcb
                else:
                    wc = sm_pool.tile([128, LT], f32, name=f"wcf_{b}", tag="wcf")
                    nc.vector.tensor_copy(wc, wT_ps)
                    state["wc"] = wc

            def d2_ctx(half):
                if last:
                    # kernel tail: PE is idle here, and the VectorE tree would
                    # serialize — direct PE matmuls, pipelined by lt-halves so
                    # the first four weight columns start the reduction early
                    ctx_ps = state.get("ctx_ps")
                    if ctx_ps is None:
                        ctx_ps = rowps.tile(
                            [1, E], f32, tag="rowps", name=f"ctx_ps_{b}"
                        )
                        state["ctx_ps"] = ctx_ps
                    for lt in range(4 * half, 4 * half + 4):
                        for es in (0, 512):
                            nc.tensor.matmul(
                                ctx_ps[:, es:es + 512],
                                state["wcb"][:, lt:lt + 1],
                                enc_n[:, lt, es:es + 512],
                                start=(lt == 0),
                                stop=(lt == LT - 1),
                            )
                    if half == 1:
                        ctx_sb = out_pool.tile([1, E], f32, name=f"ctx_sb_{b}")
                        nc.vector.tensor_scalar_mul(ctx_sb, ctx_ps, st[:, 4:5])
                        nc.sync.dma_start(ctx_d[b:b + 1, :], ctx_sb)
                    return
                # ctx partial products on VectorE: ct_lt = w[lt-chunk] * enc_nat
                # (per-partition scalar), pairwise-summed down to one [128, E]
                # tile; the 128-partition reduction is two ones-matmuls on PE
                wc = state["wc"]
                s_tiles = []
                for i in range(4):
                    lt = half * 4 + i
                    ct = ypool.tile(
                        [128, E], bf, name=f"ct{i}", tag=f"ct{i}", bufs=1
                    )
                    nc.vector.tensor_scalar_mul(
                        ct, enc_n[:, lt, :], wc[:, lt:lt + 1]
                    )
                    s_tiles.append(ct)
                s0 = ypool.tile(
                    [128, E], bf, name=f"cs{half}", tag=f"cs{half}", bufs=1
                )
                nc.vector.tensor_add(s0, s_tiles[0], s_tiles[1])
                s1 = ypool.tile(
                    [128, E], bf, name=f"cs{half}b", tag=f"cs{half}b", bufs=1
                )
                nc.vector.tensor_add(s1, s_tiles[2], s_tiles[3])
                state[f"s{half}"] = (s0, s1)

            def d3_ctx_adds():
                if last:
                    return
                a0, a1 = state["s0"]
                b0, b1 = state["s1"]
                t0 = ypool.tile([128, E], bf, name="cty0", tag="ct0", bufs=1)
                nc.vector.tensor_add(t0, a0, a1)
                t1 = ypool.tile([128, E], bf, name="cty1", tag="ct1", bufs=1)
                nc.vector.tensor_add(t1, b0, b1)
                cty = ypool.tile([128, E], bf, name="cty", tag="ct2", bufs=1)
                nc.vector.tensor_add(cty, t0, t1)
                state["cty"] = cty

            def d4_ctx_out():
                if last:
                    return
                cty = state["cty"]
                ctx_ps = rowps.tile(
                    [1, E], f32, tag="rowps", name=f"ctx_ps_{b}"
                )
                for ec in range(2):
                    es = ec * 512
                    nc.tensor.matmul(
                        ctx_ps[:, es:es + 512],
                        ones_col[:, :],
                        cty[:, es:es + 512],
                        start=True,
                        stop=True,
                    )
                ctx_sb = out_pool.tile([1, E], f32)
                nc.vector.tensor_scalar_mul(ctx_sb, ctx_ps, st[:, 4:5])
                nc.sync.dma_start(ctx_d[b:b + 1, :], ctx_sb)

            return [d0_att_lc1, d0b_exp, lambda: None, d0c_transpose,
                    lambda: d2_ctx(0), lambda: d2_ctx(1), d3_ctx_adds, d4_ctx_out]

        deferred = []
        enc_tb_tiles = {0: enc_tb0, 1: enc_tb1}
        enc_t8_tiles = {0: enc_t80, 1: enc_t81}
        for b in range(NB):
            nb2 = b + 2
            if nb2 < NB:
                tb = trb_pool.tile(
                    [128, KB, L], bf, tag="enc_tb", name=f"enc_tb{nb2}"
                )
                t8 = tr8_pool.tile(
                    [128, KF, L], f8, tag="enc_t8", name=f"enc_t8{nb2}"
                )
                rb = nb2 * 128
                nc.sync.dma_start(tb, enc_trb[rb:rb + 128, :])
                nc.sync.dma_start(t8, enc_tr8[rb:rb + 128, :])
                enc_tb_tiles[nb2] = tb
                enc_t8_tiles[nb2] = t8
            enc_tb = enc_tb_tiles.pop(b)
            enc_t8 = enc_t8_tiles.pop(b)

            def tb_ap(k, ls, enc_tb=enc_tb):
                return enc_tb[:, k, ls:ls + 512]

            def t8_ap(kp, ls, enc_t8=enc_t8):
                return enc_t8[:, 2 * kp:2 * kp + 2, ls:ls + 512]
            enc_n = nat_pool.tile([128, LT, E], bf)

            # y'[h, l] = |wv_h| * relu(energy) — evacuated straight from PSUM
            yt = en_pool.tile([128, HT, L], bf)
            att_ps = None
            pending = []
            gidx = 0

            def emit_evac(zp, lc, ht):
                ls = lc * 512
                nc.scalar.activation(
                    yt[:, ht, ls:ls + 512], zp, AF.Relu,
                    bias=hbw_sb[:, ht, b:b + 1], scale=wvs_sb[:, ht:ht + 1],
                )

            if b == 0:
                # ---- warm start: lc0 in k-outer order over chunk pairs so the
                # PE consumes each k-tile as its DMA lands instead of waiting
                # for the whole contraction ----
                def fill(n):
                    for _ in range(n):
                        nc.tensor.matmul(
                            wup_ps, wup[:, :], wup[:, :], start=True, stop=True
                        )

                for pair in range(2):
                    hts = (2 * pair, 2 * pair + 1)
                    zp_pair = {
                        ht: zps.tile([128, 512], f32, name=f"zp0{ht}", tag="zp")
                        for ht in hts
                    }
                    for k in range(KB):
                        for ht in hts:
                            nc.tensor.matmul(
                                zp_pair[ht],
                                w2b_sb[:, k, ht * 128:(ht + 1) * 128],
                                enc_tb[:, k, 0:512],
                                start=(k == 0),
                                stop=False,
                            )
                        if pair == 0:
                            fill(8)
                    for kp in range(KF // 2):
                        for ht in hts:
                            nc.tensor.matmul(
                                zp_pair[ht],
                                w28_sb[:, 2 * kp:2 * kp + 2,
                                       ht * 128:(ht + 1) * 128],
                                enc_t8[:, 2 * kp:2 * kp + 2, 0:512],
                                start=False,
                                stop=(kp == KF // 2 - 1),
                                perf_mode=DR,
                            )
                    if pair == 0:
                        # hb weights (w1/hidT) have landed by now; trace it
                        # before the first evacuation reads hbw
                        emit_hb()
                    for ht in hts:
                        emit_evac(zp_pair[ht], 0, ht)
                nc.scalar.dma_start(enc_n, enc_nat[0:128, :])

            for lc in range(2):
                if b == 0 and lc == 0:
                    continue
                ls = lc * 512
                for ht in range(HT):
                    zp = zps.tile([128, 512], f32, tag="zp")
                    for k in range(KB):
                        nc.tensor.matmul(
                            zp,
                            w2b_sb[:, k, ht * 128:(ht + 1) * 128],
                            tb_ap(k, ls),
                            start=(k == 0),
                            stop=False,
                        )
                    for kp in range(KF // 2):
                        nc.tensor.matmul(
                            zp,
                            w28_sb[:, 2 * kp:2 * kp + 2,
                                   ht * 128:(ht + 1) * 128],
                            t8_ap(kp, ls),
                            start=False,
                            stop=(kp == KF // 2 - 1),
                            perf_mode=DR,
                        )
                    emit_evac(zp, lc, ht)
                    if lc == 0 and ht == 3:
                        # natural-layout load issued mid-batch on the second
                        # HWDGE ring: needed only by ctx during the next batch,
                        # and issuing it late keeps the z path fed first
                        nc.scalar.dma_start(
                            enc_n, enc_nat[b * 128:(b + 1) * 128, :]
                        )
                    # previous batch's deferred att/softmax/ctx work slots in
                    # between this batch's z groups
                    if gidx < len(deferred):
                        deferred[gidx]()
                    gidx += 1
                    if lc == 1 and ht == 0:
                        # this batch's att(lc0): slack after the lc0 evacuations
                        att_ps = rowps.tile([1, L], f32, tag="rowps")
                        for ht2 in range(HT):
                            nc.tensor.matmul(
                                att_ps[:, 0:512],
                                sgn_sb[:, ht2:ht2 + 1],
                                yt[:, ht2, 0:512],
                                start=(ht2 == 0),
                                stop=(ht2 == HT - 1),
                            )
            deferred = make_deferred(b, att_ps, enc_n, yt,
                                     last=(b == NB - 1))

        # drain the last batch's deferred work
        for fn in deferred:
            fn()

    nc.compile()
    return nc


def _get_program():
    if "nc" not in _CACHE:
        _CACHE["nc"] = _build_program()
    return _CACHE["nc"]


def _pmajor(a, tiles, p=128):
    """[tiles*p, F] -> [p, tiles*F] partition-major packing."""
    t, rem = divmod(a.shape[0], p)
    assert rem == 0 and t == tiles
    f = a.shape[1]
    return np.ascontiguousarray(
        a.reshape(tiles, p, f).transpose(1, 0, 2).reshape(p, tiles * f)
    )


def _prep_in_maps(hidden, encoder_outputs, W_attn, b_attn, W_v):
    hidden = np.asarray(hidden, dtype=np.float32)
    encoder_outputs = np.asarray(encoder_outputs, dtype=np.float32)
    W_attn = np.asarray(W_attn, dtype=np.float32)
    b_attn = np.asarray(b_attn, dtype=np.float32)
    W_v = np.asarray(W_v, dtype=np.float32)

    enc_bf = encoder_outputs.astype(BF16)
    enc_s = encoder_outputs * ENC_SCALE           # scaled copy for the z matmul
    W2s = W_attn[H:] * W2_SCALE
    w2b = _pmajor(np.ascontiguousarray(W2s[:KB * 128]).astype(BF16), KB)
    w28 = _pmajor(np.ascontiguousarray(W2s[KB * 128:]).astype(F8E4), KF)
    w1 = _pmajor(np.ascontiguousarray(W_attn[:H]).astype(BF16), HT)
    bvec = np.ascontiguousarray(b_attn.reshape(HT, 128).T)
    wv = W_v[:, 0]
    wva = np.abs(wv).astype(np.float32)
    wvs = (wva * Z_UNSCALE).astype(np.float32)
    sgn = np.where(wv >= 0, 1.0, -1.0)
    wva = np.ascontiguousarray(wva.reshape(HT, 128).T)
    wvs = np.ascontiguousarray(wvs.reshape(HT, 128).T)
    sgn = np.ascontiguousarray(sgn.reshape(HT, 128).T.astype(BF16))
    cf32 = np.ascontiguousarray(
        np.concatenate([bvec, wvs, wva], axis=1).astype(np.float32))

    in_maps = []
    for c in range(N_CORES):
        sl = slice(c * NB, (c + 1) * NB)
        eb = enc_bf[sl]
        # natural [l, e] rows, partition-major per batch: [NB*128, LT*E]
        nat = np.ascontiguousarray(
            eb.reshape(NB, LT, 128, E).transpose(0, 2, 1, 3)
        ).reshape(NB * 128, LT * E)
        # transposed [e, l] rows, partition-major per batch, split by k-tile
        # precision: bf16 tiles 0..KB-1 and fp8 tiles KB..KT-1 (both scaled)
        et = enc_s[sl].transpose(0, 2, 1)         # [NB, E, L] scaled
        trb = np.ascontiguousarray(
            et[:, :KB * 128].astype(BF16)
            .reshape(NB, KB, 128, L).transpose(0, 2, 1, 3)
        ).reshape(NB * 128, KB * L)
        tr8 = np.ascontiguousarray(
            et[:, KB * 128:].astype(F8E4)
            .reshape(NB, KF, 128, L).transpose(0, 2, 1, 3)
        ).reshape(NB * 128, KF * L)
        hidT = _pmajor(np.ascontiguousarray(hidden[sl].T).astype(BF16), HT)
        cbf = np.ascontiguousarray(
            np.concatenate([w1, hidT, sgn], axis=1).astype(BF16))
        in_maps.append({
            "enc_nat": nat,
            "enc_trb": trb,
            "enc_tr8": tr8,
            "w2b": w2b,
            "w28": w28,
            "cbf": cbf,
            "cf32": cf32,
        })
    return in_maps


def _run(inputs, trace=False, tmpdir=None):
    from concourse.bass_utils import run_bass_kernel_spmd

    nc = _get_program()
    in_maps = _prep_in_maps(**inputs)
    res = run_bass_kernel_spmd(
        nc, in_maps, core_ids=list(range(N_CORES)), trace=trace, tmpdir=tmpdir
    )
    out = np.concatenate(
        [np.asarray(res.results[c]["ctx"]) for c in range(N_CORES)], axis=0
    ).astype(np.float32)
    return out.reshape(B, 1, E), res


def kernel(hidden, encoder_outputs, W_attn, b_attn, W_v):
    out, _ = _run(dict(
        hidden=hidden, encoder_outputs=encoder_outputs,
        W_attn=W_attn, b_attn=b_attn, W_v=W_v,
    ))
    return out


# revision 54
# speedup vs baseline: 1.0440x; 1.0079x over previous
"""Trainium2 Bass kernel for nn_AttentionLayers (B=64, L=1024, H=512, E=2H=1024).

  context[b] = softmax_l( relu(cat(hidden[b], enc[b,l]) @ W_attn + b_attn) @ W_v ) @ enc[b]

Strategy (data-parallel over batch, 8 batches per core on 8 cores):
  - split W_attn into W1 (hidden part, [512,512]) and W2 (encoder part, [1024,512]).
  - per core, precompute hbT[h, b] = (hidden @ W1 + b_attn).T once on TensorE (tiny),
    then hbw = hbT * |wv| (per-h fold of the attention vector, see below).
  - per batch, the dominant matmul zT[h, l] = sum_k W2[k,h] * encT[k,l] runs in MIXED
    precision along k: the first 4 k-tiles in bf16 (1 matmul each), the last 4 k-tiles
    as fp8e4m3 with MatmulPerfMode.DoubleRow (2 k-tiles per instruction) -> 6 PE
    instructions per 512-wide chunk instead of 8.  All four operands are pre-scaled by
    powers of two (enc*16, W2*128) so both halves accumulate in one PSUM group; the
    2^-11 unscale is folded into the evacuation.  fp8 on half the contraction keeps the
    final rel err ~1.7e-2 (< 2e-2 gate; full fp8 would be 2.39e-2).
  - |wv| is folded into the PSUM evacuation (ScalarE: y' = relu(z*|wv|*2^-11 + hb*|wv|)
    with per-partition scale/bias APs), so att[1, l] is just a +-1-column matmul
    reduction on the PE: att = sum_ht sgn_ht^T @ y'_ht.  This removes the whole
    VectorE partial-product tree of the previous version.
  - softmax: logits are bounded (|att| < ~8) so exp runs WITHOUT the usual
    max-subtraction (ScalarE exp+accum straight from PSUM); w transposed to
    columns via PE transposes.
  - ctx[1, e] = sum_l w[l] * enc_nat[l, e]: VectorE scale+tree over the natural-layout
    bf16 copy of enc, with the 128-partition reduction as ones-matmuls on the PE
    (direct PE matmuls for the last batches to shorten the kernel tail).
  - enc is supplied from the host in BOTH layouts, pre-packed partition-major:
    transposed [e, l] split into a bf16 half (k-tiles 0-3, scaled x16) and an fp8 half
    (k-tiles 4-7, scaled x16), and natural [l, e] in bf16 (unscaled, for ctx).
"""

import sys

for _p in ("/opt/trn_rl_repo",):
    if _p not in sys.path:
        sys.path.insert(0, _p)

import numpy as np
import ml_dtypes

BF16 = ml_dtypes.bfloat16
F8E4 = ml_dtypes.float8_e4m3

N_CORES = 8
B, L, H = 64, 1024, 512
E = 2 * H            # 1024
NB = B // N_CORES    # 8 batches per core
KT = E // 128        # 8 k-tiles over encoder feature dim
KB = 4               # k-tiles 0..KB-1 in bf16
KF = KT - KB         # k-tiles KB..KT-1 in fp8 (DoubleRow pairs)
HT = H // 128        # 4 tiles over hidden dim
LT = L // 128        # 8 l-tiles

# power-of-two scales (exact in bf16/fp8) for the z matmul operands
ENC_SCALE = 16.0     # |enc| <= ~6   -> |enc*16| <= ~96  < 240 (e4m3 max)
W2_SCALE = 128.0     # |W2| <= ~0.15 -> |W2*128| <= ~20  < 240
Z_UNSCALE = 1.0 / (ENC_SCALE * W2_SCALE)  # 2^-11, folded into the evac scale

_CACHE = {}


def _build_program():
    import concourse.tile as tile
    from concourse import bacc, mybir
    from contextlib import ExitStack

    f32 = mybir.dt.float32
    bf = mybir.dt.bfloat16
    f8 = mybir.dt.float8e4
    AF = mybir.ActivationFunctionType
    DR = mybir.MatmulPerfMode.DoubleRow

    nc = bacc.Bacc("TRN2", target_bir_lowering=False, debug=False, enable_asserts=False)

    # all inputs are packed partition-major on the host: row p holds everything
    # partition p needs, contiguously.
    enc_nat = nc.dram_tensor("enc_nat", [NB * 128, LT * E], bf, kind="ExternalInput").ap()
    enc_trb = nc.dram_tensor("enc_trb", [NB * 128, KB * L], bf, kind="ExternalInput").ap()
    enc_tr8 = nc.dram_tensor("enc_tr8", [NB * 128, KF * L], f8, kind="ExternalInput").ap()
    w2b_d = nc.dram_tensor("w2b", [128, KB * H], bf, kind="ExternalInput").ap()
    w28_d = nc.dram_tensor("w28", [128, KF * H], f8, kind="ExternalInput").ap()
    # merged small consts: [w1 | hidT | sgn] bf16 and [bvec | wvs | wva] f32
    cb_d = nc.dram_tensor(
        "cbf", [128, HT * H + HT * NB + HT], bf, kind="ExternalInput").ap()
    cf_d = nc.dram_tensor("cf32", [128, 3 * HT], f32, kind="ExternalInput").ap()
    ctx_d = nc.dram_tensor("ctx", [NB, E], f32, kind="ExternalOutput").ap()

    with tile.TileContext(nc) as tc, ExitStack() as ctx:
        consts = ctx.enter_context(tc.tile_pool(name="consts", bufs=1))
        nat_pool = ctx.enter_context(tc.tile_pool(name="nat", bufs=2))
        trb_pool = ctx.enter_context(tc.tile_pool(name="trb", bufs=3))
        tr8_pool = ctx.enter_context(tc.tile_pool(name="tr8", bufs=3))
        en_pool = ctx.enter_context(tc.tile_pool(name="en", bufs=2))
        sm_pool = ctx.enter_context(tc.tile_pool(name="sm", bufs=2))
        out_pool = ctx.enter_context(tc.tile_pool(name="outp", bufs=2))
        ypool = ctx.enter_context(tc.tile_pool(name="ypool", bufs=2))
        zps = ctx.enter_context(tc.tile_pool(name="zps", bufs=2, space="PSUM"))
        rowps = ctx.enter_context(tc.tile_pool(name="rowps", bufs=2, space="PSUM"))
        smallps = ctx.enter_context(tc.tile_pool(name="smallps", bufs=1, space="PSUM"))

        # ---- PE warm-up: dummy matmuls with no DMA deps keep the PE busy while
        # the first loads land, so HAM un-throttles before the real work ----
        N_WARMUP = 14
        wup = consts.tile([128, 128], bf)
        nc.vector.memset(wup[:, :], 0.0)
        wup_ps = smallps.tile([128, 128], f32, tag="wup")
        for _ in range(N_WARMUP):
            nc.tensor.matmul(wup_ps, wup[:, :], wup[:, :], start=True, stop=True)

        # ---- startup loads: interleave W2 blocks with batch-0 enc k-tiles in
        # consumption order so the z matmuls can start as soon as possible ----
        w2b_sb = consts.tile([128, KB, H], bf)
        w28_sb = consts.tile([128, KF, H], f8)
        enc_tb0 = trb_pool.tile([128, KB, L], bf, tag="enc_tb")
        enc_t80 = tr8_pool.tile([128, KF, L], f8, tag="enc_t8")

        # sync ring: batch-0 tiles in k-outer consumption order, few large
        # issues (each dma_start costs ~0.7us of issue time on its ring)
        nc.sync.dma_start(w2b_sb[:, 0, :], w2b_d[:, 0:H])
        nc.sync.dma_start(enc_tb0[:, 0, :], enc_trb[0:128, 0:L])
        nc.sync.dma_start(w2b_sb[:, 1:KB, :], w2b_d[:, H:KB * H])
        nc.sync.dma_start(enc_tb0[:, 1, :], enc_trb[0:128, L:2 * L])
        nc.sync.dma_start(enc_tb0[:, 2, :], enc_trb[0:128, 2 * L:3 * L])
        nc.sync.dma_start(enc_tb0[:, 3, :], enc_trb[0:128, 3 * L:4 * L])
        nc.sync.dma_start(w28_sb, w28_d[:, :])
        nc.sync.dma_start(enc_t80[:, 0:2, :], enc_tr8[0:128, 0:2 * L])
        nc.sync.dma_start(enc_t80[:, 2:4, :], enc_tr8[0:128, 2 * L:4 * L])
        # scalar ring (idle this early): merged small consts, then batch-1
        cf_sb = consts.tile([128, 3 * HT], f32)
        nc.scalar.dma_start(cf_sb, cf_d[:, :])
        cb_sb = consts.tile([128, HT * H + HT * NB + HT], bf)
        nc.scalar.dma_start(cb_sb, cb_d[:, :])
        w1_sb = cb_sb[:, 0:HT * H].rearrange("p (t h) -> p t h", t=HT)
        hidT_sb = cb_sb[:, HT * H:HT * H + HT * NB].rearrange(
            "p (t n) -> p t n", t=HT)
        sgn_sb = cb_sb[:, HT * H + HT * NB:]
        b_sb = cf_sb[:, 0:HT]
        wvs_sb = cf_sb[:, HT:2 * HT]
        wva_sb = cf_sb[:, 2 * HT:3 * HT]
        enc_tb1 = trb_pool.tile([128, KB, L], bf, tag="enc_tb", name="enc_tb1")
        nc.scalar.dma_start(enc_tb1, enc_trb[128:256, :])
        enc_t81 = tr8_pool.tile([128, KF, L], f8, tag="enc_t8", name="enc_t81")
        nc.scalar.dma_start(enc_t81, enc_tr8[128:256, :])
        ident = consts.tile([1, 1], f32)
        nc.vector.memset(ident[:, :], 1.0)
        ones_col = consts.tile([128, 1], bf)
        nc.vector.memset(ones_col[:, :], 1.0)

        # hbw[h, b] = (hidden @ W1 + b_attn).T * |wv|  — emitted lazily (after a
        # couple of batch-0 z groups) so its weight loads don't stall the PE stream.
        hbw_sb = consts.tile([128, HT, NB], f32)

        def emit_hb():
            for ht in range(HT):
                hb_ps = smallps.tile([128, NB], f32, tag="hb")
                for k in range(HT):
                    nc.tensor.matmul(
                        hb_ps,
                        w1_sb[:, k, ht * 128:(ht + 1) * 128],
                        hidT_sb[:, k, :],
                        start=(k == 0),
                        stop=(k == HT - 1),
                    )
                nc.scalar.activation(
                    hbw_sb[:, ht, :], hb_ps, AF.Identity,
                    bias=b_sb[:, ht:ht + 1], scale=1.0,
                )
                nc.vector.tensor_scalar_mul(
                    hbw_sb[:, ht, :], hbw_sb[:, ht, :], wva_sb[:, ht:ht + 1]
                )

        # ---- per-batch pipeline, software-pipelined across batches:
        # batch b's att(lc1) / softmax / w-transposes / ctx work is deferred and
        # spread over several slots between batch b+1's z groups so no PE
        # instruction ever sits in the queue behind unfinished Scalar/DVE work ----
        def make_deferred(b, att_ps, enc_n, yt, last):
            final = b == NB - 1

            def fillers(n):
                for _ in range(n):
                    nc.tensor.matmul(
                        wup_ps, wup[:, :], wup[:, :], start=True, stop=True
                    )

            st = sm_pool.tile([1, 5], f32, tag="softstate", name=f"st_{b}")
            # [1:3]=exp partial sums, [3]=sumexp, [4]=1/sumexp
            w_row = sm_pool.tile([1, L], f32, name=f"w_row_{b}")
            state = {}

            def d0_att_lc1():
                if final:
                    fillers(10)
                for ht in range(HT):
                    nc.tensor.matmul(
                        att_ps[:, 512:1024],
                        sgn_sb[:, ht:ht + 1],
                        yt[:, ht, 512:1024],
                        start=(ht == 0),
                        stop=(ht == HT - 1),
                    )

            def d0b_exp():
                # logits are bounded (|att| < ~8): f32 exp is safe without the
                # usual max-subtraction, which keeps VectorE off the chain
                for lc2 in range(2):
                    ls2 = lc2 * 512
                    nc.scalar.activation(
                        w_row[:, ls2:ls2 + 512], att_ps[:, ls2:ls2 + 512], AF.Exp,
                        bias=0.0, scale=1.0,
                        accum_out=st[:, 1 + lc2:2 + lc2],
                    )
                nc.vector.tensor_reduce(
                    st[:, 3:4], st[:, 1:3], axis=mybir.AxisListType.X,
                    op=mybir.AluOpType.add,
                )
                nc.vector.reciprocal(st[:, 4:5], st[:, 3:4])

            def d0c_transpose():
                if final:
                    fillers(16)
                wT_ps = smallps.tile([128, LT], f32, tag="wup")
                for j in range(8):
                    nc.tensor.transpose(
                        wT_ps[:, j:j + 1], w_row[:, j * 128:(j + 1) * 128],
                        ident[:, :],
                    )
                if last:
                    wcb = sm_pool.tile([128, LT], bf, name=f"wcb_{b}", tag="wcb")
                    nc.vector.tensor_copy(wcb[:, 0:4], wT_ps[:, 0:4])
                    nc.vector.tensor_copy(wcb[:, 4:8], wT_ps[:, 4:8])
                    state["wcb"] = wcb
                else:
                    wc = sm_pool.tile([128, LT], f32, name=f"wcf_{b}", tag="wcf")
                    nc.vector.tensor_copy(wc, wT_ps)
                    state["wc"] = wc

            def d2_ctx(half):
                if last:
                    # kernel tail: PE is idle here, and the VectorE tree would
                    # serialize — direct PE matmuls, pipelined by lt-halves so
                    # the first four weight columns start the reduction early
                    ctx_ps = state.get("ctx_ps")
                    if ctx_ps is None:
                        ctx_ps = rowps.tile(
                            [1, E], f32, tag="rowps", name=f"ctx_ps_{b}"
                        )
                        state["ctx_ps"] = ctx_ps
                    for lt in range(4 * half, 4 * half + 4):
                        for es in (0, 512):
                            nc.tensor.matmul(
                                ctx_ps[:, es:es + 512],
                                state["wcb"][:, lt:lt + 1],
                                enc_n[:, lt, es:es + 512],
                                start=(lt == 0),
                                stop=(lt == LT - 1),
                            )
                    if half == 1:
                        ctx_sb = out_pool.tile([1, E], f32, name=f"ctx_sb_{b}")
                        nc.vector.tensor_scalar_mul(ctx_sb, ctx_ps, st[:, 4:5])
                        nc.sync.dma_start(ctx_d[b:b + 1, :], ctx_sb)
                    return
                # ctx partial products on VectorE: ct_lt = w[lt-chunk] * enc_nat
                # (per-partition scalar), pairwise-summed down to one [128, E]
                # tile; the 128-partition reduction is two ones-matmuls on PE
                wc = state["wc"]
                s_tiles = []
                for i in range(4):
                    lt = half * 4 + i
                    ct = ypool.tile(
                        [128, E], bf, name=f"ct{i}", tag=f"ct{i}", bufs=1
                    )
                    nc.vector.tensor_scalar_mul(
                        ct, enc_n[:, lt, :], wc[:, lt:lt + 1]
                    )
                    s_tiles.append(ct)
                s0 = ypool.tile(
                    [128, E], bf, name=f"cs{half}", tag=f"cs{half}", bufs=1
                )
                nc.vector.tensor_add(s0, s_tiles[0], s_tiles[1])
                s1 = ypool.tile(
                    [128, E], bf, name=f"cs{half}b", tag=f"cs{half}b", bufs=1
                )
                nc.vector.tensor_add(s1, s_tiles[2], s_tiles[3])
                state[f"s{half}"] = (s0, s1)

            def d3_ctx_adds():
                if last:
                    return
                a0, a1 = state["s0"]
                b0, b1 = state["s1"]
                t0 = ypool.tile([128, E], bf, name="cty0", tag="ct0", bufs=1)
                nc.vector.tensor_add(t0, a0, a1)
                t1 = ypool.tile([128, E], bf, name="cty1", tag="ct1", bufs=1)
                nc.vector.tensor_add(t1, b0, b1)
                cty = ypool.tile([128, E], bf, name="cty", tag="ct2", bufs=1)
                nc.vector.tensor_add(cty, t0, t1)
                state["cty"] = cty

            def d4_ctx_out():
                if last:
                    return
                cty = state["cty"]
                ctx_ps = rowps.tile(
                    [1, E], f32, tag="rowps", name=f"ctx_ps_{b}"
                )
                for ec in range(2):
                    es = ec * 512
                    nc.tensor.matmul(
                        ctx_ps[:, es:es + 512],
                        ones_col[:, :],
                        cty[:, es:es + 512],
                        start=True,
                        stop=True,
                    )
                ctx_sb = out_pool.tile([1, E], f32)
                nc.vector.tensor_scalar_mul(ctx_sb, ctx_ps, st[:, 4:5])
                nc.sync.dma_start(ctx_d[b:b + 1, :], ctx_sb)

            return [d0_att_lc1, d0b_exp, lambda: None, d0c_transpose,
                    lambda: d2_ctx(0), lambda: d2_ctx(1), d3_ctx_adds, d4_ctx_out]

        deferred = []
        enc_tb_tiles = {0: enc_tb0, 1: enc_tb1}
        enc_t8_tiles = {0: enc_t80, 1: enc_t81}
        for b in range(NB):
            nb2 = b + 2
            if nb2 < NB:
                tb = trb_pool.tile(
                    [128, KB, L], bf, tag="enc_tb", name=f"enc_tb{nb2}"
                )
                t8 = tr8_pool.tile(
                    [128, KF, L], f8, tag="enc_t8", name=f"enc_t8{nb2}"
                )
                rb = nb2 * 128
                nc.sync.dma_start(tb, enc_trb[rb:rb + 128, :])
                nc.sync.dma_start(t8, enc_tr8[rb:rb + 128, :])
                enc_tb_tiles[nb2] = tb
                enc_t8_tiles[nb2] = t8
            enc_tb = enc_tb_tiles.pop(b)
            enc_t8 = enc_t8_tiles.pop(b)

            def tb_ap(k, ls, enc_tb=enc_tb):
                return enc_tb[:, k, ls:ls + 512]

            def t8_ap(kp, ls, enc_t8=enc_t8):
                return enc_t8[:, 2 * kp:2 * kp + 2, ls:ls + 512]
            enc_n = nat_pool.tile([128, LT, E], bf)

            # y'[h, l] = |wv_h| * relu(energy) — evacuated straight from PSUM
            yt = en_pool.tile([128, HT, L], bf)
            att_ps = None
            pending = []
            gidx = 0

            def emit_evac(zp, lc, ht):
                ls = lc * 512
                nc.scalar.activation(
                    yt[:, ht, ls:ls + 512], zp, AF.Relu,
                    bias=hbw_sb[:, ht, b:b + 1], scale=wvs_sb[:, ht:ht + 1],
                )

            if b == 0:
                # ---- warm start: lc0 in k-outer order over chunk pairs so the
                # PE consumes each k-tile as its DMA lands instead of waiting
                # for the whole contraction ----
                def fill(n):
                    for _ in range(n):
                        nc.tensor.matmul(
                            wup_ps, wup[:, :], wup[:, :], start=True, stop=True
                        )

                for pair in range(2):
                    hts = (2 * pair, 2 * pair + 1)
                    zp_pair = {
                        ht: zps.tile([128, 512], f32, name=f"zp0{ht}", tag="zp")
                        for ht in hts
                    }
                    for k in range(KB):
                        for ht in hts:
                            nc.tensor.matmul(
                                zp_pair[ht],
                                w2b_sb[:, k, ht * 128:(ht + 1) * 128],
                                enc_tb[:, k, 0:512],
                                start=(k == 0),
                                stop=False,
                            )
                        if pair == 0:
                            fill(8)
                    for kp in range(KF // 2):
                        for ht in hts:
                            nc.tensor.matmul(
                                zp_pair[ht],
                                w28_sb[:, 2 * kp:2 * kp + 2,
                                       ht * 128:(ht + 1) * 128],
                                enc_t8[:, 2 * kp:2 * kp + 2, 0:512],
                                start=False,
                                stop=(kp == KF // 2 - 1),
                                perf_mode=DR,
                            )
                    if pair == 0:
                        # hb weights (w1/hidT) have landed by now; trace it
                        # before the first evacuation reads hbw
                        emit_hb()
                    for ht in hts:
                        emit_evac(zp_pair[ht], 0, ht)
                nc.scalar.dma_start(enc_n, enc_nat[0:128, :])

            for lc in range(2):
                if b == 0 and lc == 0:
                    continue
                ls = lc * 512
                for ht in range(HT):
                    zp = zps.tile([128, 512], f32, tag="zp")
                    for k in range(KB):
                        nc.tensor.matmul(
                            zp,
                            w2b_sb[:, k, ht * 128:(ht + 1) * 128],
                            tb_ap(k, ls),
                            start=(k == 0),
                            stop=False,
                        )
                    for kp in range(KF // 2):
                        nc.tensor.matmul(
                            zp,
                            w28_sb[:, 2 * kp:2 * kp + 2,
                                   ht * 128:(ht + 1) * 128],
                            t8_ap(kp, ls),
                            start=False,
                            stop=(kp == KF // 2 - 1),
                            perf_mode=DR,
                        )
                    emit_evac(zp, lc, ht)
                    if lc == 0 and ht == 3:
                        # natural-layout load issued mid-batch on the second
                        # HWDGE ring: needed only by ctx during the next batch,
                        # and issuing it late keeps the z path fed first
                        nc.scalar.dma_start(
                            enc_n, enc_nat[b * 128:(b + 1) * 128, :]
                        )
                    # previous batch's deferred att/softmax/ctx work slots in
                    # between this batch's z groups
                    if gidx < len(deferred):
                        deferred[gidx]()
                    gidx += 1
                    if lc == 1 and ht == 0:
                        # this batch's att(lc0): slack after the lc0 evacuations
                        att_ps = rowps.tile([1, L], f32, tag="rowps")
                        for ht2 in range(HT):
                            nc.tensor.matmul(
                                att_ps[:, 0:512],
                                sgn_sb[:, ht2:ht2 + 1],
                                yt[:, ht2, 0:512],
                                start=(ht2 == 0),
                                stop=(ht2 == HT - 1),
                            )
            deferred = make_deferred(b, att_ps, enc_n, yt,
                                     last=(b == NB - 1))

        # drain the last batch's deferred work
        for fn in deferred:
            fn()

    nc.compile()
    return nc


def _get_program():
    if "nc" not in _CACHE:
        _CACHE["nc"] = _build_program()
    return _CACHE["nc"]


def _pmajor(a, tiles, p=128):
    """[tiles*p, F] -> [p, tiles*F] partition-major packing."""
    t, rem = divmod(a.shape[0], p)
    assert rem == 0 and t == tiles
    f = a.shape[1]
    return np.ascontiguousarray(
        a.reshape(tiles, p, f).transpose(1, 0, 2).reshape(p, tiles * f)
    )


def _prep_in_maps(hidden, encoder_outputs, W_attn, b_attn, W_v):
    hidden = np.asarray(hidden, dtype=np.float32)
    encoder_outputs = np.asarray(encoder_outputs, dtype=np.float32)
    W_attn = np.asarray(W_attn, dtype=np.float32)
    b_attn = np.asarray(b_attn, dtype=np.float32)
    W_v = np.asarray(W_v, dtype=np.float32)

    enc_bf = encoder_outputs.astype(BF16)
    enc_s = encoder_outputs * ENC_SCALE           # scaled copy for the z matmul
    W2s = W_attn[H:] * W2_SCALE
    w2b = _pmajor(np.ascontiguousarray(W2s[:KB * 128]).astype(BF16), KB)
    w28 = _pmajor(np.ascontiguousarray(W2s[KB * 128:]).astype(F8E4), KF)
    w1 = _pmajor(np.ascontiguousarray(W_attn[:H]).astype(BF16), HT)
    bvec = np.ascontiguousarray(b_attn.reshape(HT, 128).T)
    wv = W_v[:, 0]
    wva = np.abs(wv).astype(np.float32)
    wvs = (wva * Z_UNSCALE).astype(np.float32)
    sgn = np.where(wv >= 0, 1.0, -1.0)
    wva = np.ascontiguousarray(wva.reshape(HT, 128).T)
    wvs = np.ascontiguousarray(wvs.reshape(HT, 128).T)
    sgn = np.ascontiguousarray(sgn.reshape(HT, 128).T.astype(BF16))
    cf32 = np.ascontiguousarray(
        np.concatenate([bvec, wvs, wva], axis=1).astype(np.float32))

    in_maps = []
    for c in range(N_CORES):
        sl = slice(c * NB, (c + 1) * NB)
        eb = enc_bf[sl]
        # natural [l, e] rows, partition-major per batch: [NB*128, LT*E]
        nat = np.ascontiguousarray(
            eb.reshape(NB, LT, 128, E).transpose(0, 2, 1, 3)
        ).reshape(NB * 128, LT * E)
        # transposed [e, l] rows, partition-major per batch, split by k-tile
        # precision: bf16 tiles 0..KB-1 and fp8 tiles KB..KT-1 (both scaled)
        et = enc_s[sl].transpose(0, 2, 1)         # [NB, E, L] scaled
        trb = np.ascontiguousarray(
            et[:, :KB * 128].astype(BF16)
            .reshape(NB, KB, 128, L).transpose(0, 2, 1, 3)
        ).reshape(NB * 128, KB * L)
        tr8 = np.ascontiguousarray(
            et[:, KB * 128:].astype(F8E4)
            .reshape(NB, KF, 128, L).transpose(0, 2, 1, 3)
        ).reshape(NB * 128, KF * L)
        hidT = _pmajor(np.ascontiguousarray(hidden[sl].T).astype(BF16), HT)
        cbf = np.ascontiguousarray(
            np.concatenate([w1, hidT, sgn], axis=1).astype(BF16))
        in_maps.append({
            "enc_nat": nat,
            "enc_trb": trb,
            "enc_tr8": tr8,
            "w2b": w2b,
            "w28": w28,
            "cbf": cbf,
            "cf32": cf32,
        })
    return in_maps


def _run(inputs, trace=False, tmpdir=None):
    from concourse.bass_utils import run_bass_kernel_spmd

    nc = _get_program()
    in_maps = _prep_in_maps(**inputs)
    res = run_bass_kernel_spmd(
        nc, in_maps, core_ids=list(range(N_CORES)), trace=trace, tmpdir=tmpdir
    )
    out = np.concatenate(
        [np.asarray(res.results[c]["ctx"]) for c in range(N_CORES)], axis=0
    ).astype(np.float32)
    return out.reshape(B, 1, E), res


def kernel(hidden, encoder_outputs, W_attn, b_attn, W_v):
    out, _ = _run(dict(
        hidden=hidden, encoder_outputs=encoder_outputs,
        W_attn=W_attn, b_attn=b_attn, W_v=W_v,
    ))
    return out
